# revision 18
# baseline (speedup 1.0000x reference)
"""AttentionConv kernel for Trainium2 (8 NeuronCores, SPMD data-parallel over batch).

Problem: per-channel windowed softmax attention.
  q = wq @ x; k = wk @ pad(x, 3); v = wv @ pad(x, 3)       (1x1 convs = GEMMs)
  s_j[c,w] = q[c,w] * k[c,w+j],  j = 0..6
  out[c,w] = sum_j softmax_j(s)[c,w,j] * v[c,w+j]

Sharding: batch B=8 -> one batch element per core; weights replicated.

v2 engine mapping (vs v1 which ran everything elementwise on DVE/ACT):
  TensorE: q/k/v GEMMs (bf16) AND the two 7-plane window reductions
           (den = sum_j e_j, num = sum_j e_j*v_j) as identity-weight
           matmuls accumulating in PSUM fp32. Keeps PE busy -> HAM stays
           at 8/8 (2.4 GHz) instead of the 4/8 throttle v1 suffered.
  VectorE: score mults and e*v mults (bf16 2x, full-width 4096 rows to
           amortize the ~235-cycle per-row bubble), exp via Schraudolph
           bit-trick tensor_scalar -> int16 (4x mode), final
           out = num(PSUM) * rden.
  ScalarE: PSUM->SBUF GEMM evacuation casts, optional exact exp planes,
           rden = exp(-ln(den)) pinned to the one ACT table set with both.
  Host upcasts the bf16 output to fp32.
"""

import sys

sys.path.insert(0, "/opt/trn_rl_repo")

import numpy as np

B, C, W = 8, 256, 4096
K7, PAD = 7, 3
WP = W + 2 * PAD
GG = 1024  # gemm / psum evac group (2 PSUM banks)
SC = 1024  # sum-chunk width for den/num PSUM accumulators

# --- tuning knobs -----------------------------------------------------------
N_ACT_PLANES = 2  # j-planes [0, n) get exact ACT exp; rest Schraudolph on DVE
SCHRAUD_C0 = 184.6650390625  # 2^7 / ln 2
SCHRAUD_C1 = 16250.0  # 127 * 128 - sigma
LDW_SKIP = True  # non-self-loading matmuls when PE weights are unchanged
FINAL_ON_POOL = True  # out = num*rden on gpsimd (ACT evacuates num first)

_STATE = {}


def _patch_act_tables():
    """Force Exp and Ln to resolve to the one ACT table set containing both,
    so the kernel pays a single ACT_TABLE_LOAD instead of thrashing."""
    import concourse.bacc as bacc_mod
    import concourse.mybir as mybir
    from concourse.hw_specs import get_activation_tables as orig

    AF = mybir.ActivationFunctionType

    def patched(arch):
        out = {}
        for name, funcs in orig(arch).items():
            f = set(funcs)
            if name != "natural_log_exp_and_others":
                f.discard(AF.Exp)
                f.discard(AF.Ln)
            out[name] = f
        return out

    bacc_mod.get_activation_tables = patched


def _build_nc():
    import concourse.bass as bass
    import concourse.tile as tile
    from concourse import bacc, mybir

    _patch_act_tables()
    _STATE.pop("pe_wkey", None)

    bf16 = mybir.dt.bfloat16
    i16 = mybir.dt.int16
    f32 = mybir.dt.float32
    AF = mybir.ActivationFunctionType
    ALU = mybir.AluOpType

    nc = bacc.Bacc("TRN2", target_bir_lowering=False, debug=False, num_devices=8)

    x_d = nc.declare_dram_parameter("x", [C, W], bf16, isOutput=False)
    w_d = {
        t: nc.declare_dram_parameter(f"wt{t}", [C, C], bf16, isOutput=False)
        for t in "qkv"
    }
    id_d = nc.declare_dram_parameter("ident", [128, 128], bf16, isOutput=False)
    out_d = nc.declare_dram_parameter("out", [C, W], bf16, isOutput=True)

    n_gg = W // GG  # gemm groups per co block (4)
    n_sc = W // SC  # sum chunks per co block (4)

    with tile.TileContext(nc) as tc:
        from contextlib import ExitStack

        with ExitStack() as ctx:
            persist = ctx.enter_context(tc.tile_pool(name="persist", bufs=1))
            lpool = ctx.enter_context(tc.tile_pool(name="lpool", bufs=2))
            rpool = ctx.enter_context(tc.tile_pool(name="rpool", bufs=3))
            npool = ctx.enter_context(tc.tile_pool(name="npool", bufs=3))
            opool = ctx.enter_context(tc.tile_pool(name="opool", bufs=3))

            # ---- persistent SBUF tensors ----
            xb = persist.tile([128, 2, W], bf16, tag="xb")  # x, ci-major blocks
            wsb = {
                t: persist.tile([128, 2, C], bf16, name=f"wsb_{t}", tag=f"wsb_{t}")
                for t in "qkv"
            }  # w.T
            idt = persist.tile([128, 128], bf16, tag="idt")
            qsb = persist.tile([128, 2, W], bf16, tag="qsb")
            ksb = persist.tile([128, 2, WP], bf16, tag="ksb")
            vsb = persist.tile([128, 2, WP], bf16, tag="vsb")
            # score/e/ev planes, both co blocks (ev done in place over e)
            st = persist.tile([128, 2, K7, W], bf16, tag="st")

            # ---- loads ----
            for cb in range(2):
                nc.sync.dma_start(
                    out=wsb["q"][:, cb, :], in_=w_d["q"][cb * 128 : (cb + 1) * 128, :]
                )
            nc.sync.dma_start(out=idt[:, :], in_=id_d[:, :])
            for cb in range(2):
                nc.sync.dma_start(
                    out=xb[:, cb, :], in_=x_d[cb * 128 : (cb + 1) * 128, :]
                )
            for t in "kv":
                for cb in range(2):
                    nc.sync.dma_start(
                        out=wsb[t][:, cb, :], in_=w_d[t][cb * 128 : (cb + 1) * 128, :]
                    )

            # zero the pad columns of k and v
            for buf in (ksb, vsb):
                for cb in range(2):
                    nc.vector.memset(buf[:, cb, 0:PAD], 0.0)
                    nc.vector.memset(buf[:, cb, W + PAD : WP], 0.0)

            def mm(out, lhsT, rhs, start, stop, wkey, epoch=False, **kw):
                """plain matmul; redundant LDWEIGHTS are removed post-schedule
                by _dedupe_ldweights (wkey/epoch kept for readability)."""
                return nc.tensor.matmul(out, lhsT, rhs, start=start, stop=stop, **kw)

            def warmup(gpsum):
                """PE warmup burst: dummy matmuls on the wq tiles so the pstate
                ramp + HAM clock-gate release before the real GEMM stream."""
                wps = gpsum.tile([128, GG], f32, name="wps", tag="gps")
                for i in range(16):
                    mm(
                        wps[:, 0:256],
                        wsb["q"][:, 0, 0:128],
                        wsb["q"][:, i % 2, :],
                        start=True,
                        stop=True,
                        wkey="warm",
                        skip_group_check=True,
                    )

            def gemm_group(co, g, t, gpsum):
                """GEMM of tensor t for output cols [g*GG, (g+1)*GG) of
                co-block, one PSUM tile + one ACT evacuation. ci-outer so the
                two 512-halves of a group share one weight load."""
                co_sl = slice(co * 128, (co + 1) * 128)
                ps = gpsum.tile([128, GG], f32, name="ps", tag="gps")
                for ci in range(2):
                    for i in range(GG // 512):
                        w0 = g * GG + i * 512
                        mm(
                            ps[:, i * 512 : (i + 1) * 512],
                            wsb[t][:, ci, co_sl],
                            xb[:, ci, w0 : w0 + 512],
                            start=(ci == 0),
                            stop=(ci == 1),
                            wkey=("w", t, co, ci),
                            epoch=(ci == 0 and i == 0),
                        )
                if t == "q":
                    dst = qsb[:, co, g * GG : (g + 1) * GG]
                else:
                    buf = ksb if t == "k" else vsb
                    dst = buf[:, co, PAD + g * GG : PAD + (g + 1) * GG]
                nc.scalar.copy(out=dst, in_=ps[:, :])

            def q_bc(co, j0, nj):
                qsl = qsb[:, co, :]
                return bass.AP(
                    tensor=qsl.tensor,
                    offset=qsl.offset,
                    ap=[qsl.ap[0], [0, nj], [1, W]],
                )

            def k_win(co, j0, nj):
                ksl = ksb[:, co, :]
                return bass.AP(
                    tensor=ksl.tensor,
                    offset=ksl.offset + j0,
                    ap=[ksl.ap[0], [1, nj], [1, W]],
                )

            def v_win(co, j0, nj):
                vsl = vsb[:, co, :]
                return bass.AP(
                    tensor=vsl.tensor,
                    offset=vsl.offset + j0,
                    ap=[vsl.ap[0], [1, nj], [1, W]],
                )

            def scores_exp(co):
                """s_j = q*k_j then e = exp(s) in place, split in j-halves so
                exp overlaps the second score mult."""
                for j0, nj in ((0, 4), (4, 3)):
                    dst = st[:, co, j0 : j0 + nj, :]
                    nc.vector.tensor_tensor(
                        dst, q_bc(co, j0, nj), k_win(co, j0, nj), ALU.mult
                    )
                    a_lo = max(j0, 0)
                    a_hi = min(j0 + nj, N_ACT_PLANES)
                    if a_hi > a_lo:  # exact ACT planes
                        sl = st[:, co, a_lo:a_hi, :]
                        nc.scalar.activation(sl, sl, AF.Exp)
                    s_lo = max(j0, N_ACT_PLANES)
                    s_hi = j0 + nj
                    if s_hi > s_lo:  # Schraudolph planes on DVE (4x)
                        sl = st[:, co, s_lo:s_hi, :]
                        nc.vector.tensor_scalar(
                            sl.bitcast(i16),
                            sl,
                            SCHRAUD_C0,
                            SCHRAUD_C1,
                            ALU.mult,
                            ALU.add,
                        )

            def ev_mult(co):
                """ev_j = e_j * v_j in place (PE den sums must already have
                consumed the e values for this co block)."""
                sl = st[:, co, :, :]
                nc.vector.tensor_tensor(sl, sl, v_win(co, 0, K7), ALU.mult)

            def pe_sum(co, m, pool, tag):
                """7-plane sum over j for w-cols [m*SC, (m+1)*SC) via identity
                matmuls accumulating in PSUM. Returns the PSUM tile."""
                ps = pool.tile([128, SC], f32, name=tag, tag=tag)
                for h in range(SC // 512):
                    w0 = m * SC + h * 512
                    for j in range(K7):
                        mm(
                            ps[:, h * 512 : (h + 1) * 512],
                            idt[:, :],
                            st[:, co, j, w0 : w0 + 512],
                            start=(j == 0),
                            stop=(j == K7 - 1),
                            wkey="ident",
                            epoch=(j == 0),
                        )
                return ps

            def rden_of(denp):
                """rden = exp(-ln(den)) on ACT; ln kept fp32 to avoid bf16
                ulp noise on large |ln den|."""
                t = lpool.tile([128, SC], f32, name="lnt", tag="lnt")
                r = rpool.tile([128, SC], bf16, name="rd", tag="rd")
                nc.scalar.activation(t[:, :], denp[:, :], AF.Ln)
                nc.scalar.activation(r[:, :], t[:, :], AF.Exp, scale=-1.0)
                return r

            def final_out(co, m, nump, rd):
                """out = num * rden -> bf16, DMA to HBM. Either DVE reads num
                straight from PSUM (1x), or ACT evacuates num to SBUF and the
                idle gpsimd does the multiply."""
                co_sl = slice(co * 128, (co + 1) * 128)
                w0 = m * SC
                oc = opool.tile([128, SC], bf16, name="oc", tag="oc")
                if FINAL_ON_POOL:
                    ns = npool.tile([128, SC], bf16, name="ns", tag="ns")
                    nc.scalar.copy(out=ns[:, :], in_=nump[:, :])
                    nc.gpsimd.tensor_tensor(oc[:, :], ns[:, :], rd[:, :], ALU.mult)
                else:
                    nc.vector.tensor_tensor(oc[:, :], nump[:, :], rd[:, :], ALU.mult)
                nc.sync.dma_start(out=out_d[co_sl, w0 : w0 + SC], in_=oc[:, :])

            # ---- emission ----
            # ACT queue order is emission order, so interleave per co-block:
            # q/k GEMM evacs, then this block's exps, then the v evacs — the
            # exp for co-block 0 must not sit behind co-block 1's evacuations.
            with tc.tile_pool(name="gpsum", bufs=3, space="PSUM") as gpsum:
                warmup(gpsum)
                for co in range(2):
                    for t in "qk":
                        for g in range(n_gg):
                            gemm_group(co, g, t, gpsum)
                    scores_exp(co)
                    for g in range(n_gg):
                        gemm_group(co, g, "v", gpsum)
            with (
                tc.tile_pool(name="dpsum", bufs=2, space="PSUM") as dpsum,
                tc.tile_pool(name="npsum", bufs=2, space="PSUM") as npsum,
            ):
                dens = {}
                for co in range(2):
                    for m in range(n_sc):
                        dens[(co, m)] = pe_sum(co, m, dpsum, "den")
                for co in range(2):
                    ev_mult(co)
                for co in range(2):
                    for m in range(n_sc):
                        rd = rden_of(dens[(co, m)])  # ACT; frees den tile
                        nump = pe_sum(co, m, npsum, "num")
                        final_out(co, m, nump, rd)

    if LDW_SKIP:
        _dedupe_ldweights(nc, mybir)
    nc.finalize()
    return nc


def _dedupe_ldweights(nc, mybir):
    """Remove redundant InstLdweights: a reload of the exact weights already
    resident in the PE array. The tile scheduler splits every matmul into
    LDWEIGHTS + MATMUL; back-to-back same-weight matmuls then pay a ~107ns
    reload plus a lost drain/fill overlap (~166ns) each. Only drops loads
    that carry no semaphore waits/updates, so sync is untouched; any other
    PE instruction type resets the tracked signature."""

    def wsig(ldw):
        return (
            str(ldw.ins[0]),
            str(ldw.is_transpose),
            str(ldw.perf_mode),
            str(ldw.tile_position),
        )

    removed = 0
    for f in nc.m.functions:
        for b in f.blocks:
            keep = []
            last = None
            for i in b.instructions:
                tn = type(i).__name__
                if getattr(i, "engine", None) == mybir.EngineType.PE:
                    if tn == "InstLdweights":
                        si = i.sync_info
                        clean = si is None or (
                            len(si.on_wait) == 0 and len(si.on_update) == 0
                        )
                        if clean and last == wsig(i):
                            removed += 1
                            continue
                        last = wsig(i)
                    elif tn in ("InstMatmult", "InstEventSemaphore"):
                        pass
                    else:
                        last = None
                keep.append(i)
            b.instructions[:] = keep
    return removed


def _get_nc():
    if "nc" not in _STATE:
        _STATE["nc"] = _build_nc()
    return _STATE["nc"]


def _make_in_maps(x, wq, wk, wv):
    import ml_dtypes

    bf = ml_dtypes.bfloat16

    x = np.asarray(x, dtype=np.float32)
    wqT = np.ascontiguousarray(np.asarray(wq, dtype=np.float32).T).astype(bf)
    wkT = np.ascontiguousarray(np.asarray(wk, dtype=np.float32).T).astype(bf)
    wvT = np.ascontiguousarray(np.asarray(wv, dtype=np.float32).T).astype(bf)
    xb = x.astype(bf)
    ident = np.eye(128, dtype=np.float32).astype(bf)

    return [
        {
            "x": np.ascontiguousarray(xb[b]),
            "wtq": wqT,
            "wtk": wkT,
            "wtv": wvT,
            "ident": ident,
        }
        for b in range(B)
    ]


def kernel(x, wq, wk, wv):
    nc = _get_nc()
    in_maps = _make_in_maps(x, wq, wk, wv)

    from concourse.bass_utils import run_bass_kernel_spmd

    res = run_bass_kernel_spmd(nc, in_maps, core_ids=list(range(B)))
    outs = [np.asarray(res.results[i]["out"], dtype=np.float32) for i in range(B)]
    return np.stack(outs)


# revision 21
# speedup vs baseline: 1.2612x; 1.2612x over previous
"""AttentionConv kernel for Trainium2 (8 NeuronCores, SPMD data-parallel over batch).

Problem: per-channel windowed softmax attention.
  q = wq @ x; k = wk @ pad(x, 3); v = wv @ pad(x, 3)       (1x1 convs = GEMMs)
  s_j[c,w] = q[c,w] * k[c,w+j],  j = 0..6
  out[c,w] = sum_j softmax_j(s)[c,w,j] * v[c,w+j]

Sharding: batch B=8 -> one batch element per core; weights replicated.

v2 engine mapping (vs v1 which ran everything elementwise on DVE/ACT):
  TensorE: q/k/v GEMMs (bf16) AND the two 7-plane window reductions
           (den = sum_j e_j, num = sum_j e_j*v_j) as identity-weight
           matmuls accumulating in PSUM fp32. Keeps PE busy -> HAM stays
           at 8/8 (2.4 GHz) instead of the 4/8 throttle v1 suffered.
  VectorE: score mults and e*v mults (bf16 2x, full-width 4096 rows to
           amortize the ~235-cycle per-row bubble), exp via Schraudolph
           bit-trick tensor_scalar -> int16 (4x mode), final
           out = num(PSUM) * rden.
  ScalarE: PSUM->SBUF GEMM evacuation casts, optional exact exp planes,
           rden = exp(-ln(den)) pinned to the one ACT table set with both.
  Host upcasts the bf16 output to fp32.
"""

import sys

sys.path.insert(0, "/opt/trn_rl_repo")

import numpy as np

B, C, W = 8, 256, 4096
K7, PAD = 7, 3
WP = W + 2 * PAD
GG = 1024  # gemm / psum evac group (2 PSUM banks)
SC = 1024  # sum-chunk width for den/num PSUM accumulators

# --- tuning knobs -----------------------------------------------------------
N_ACT_PLANES = 2  # j-planes [0, n) get exact ACT exp; rest Schraudolph on DVE
SCHRAUD_C0 = 184.6650390625  # 2^7 / ln 2
SCHRAUD_C1 = 16250.0  # 127 * 128 - sigma
LDW_SKIP = True  # non-self-loading matmuls when PE weights are unchanged
FINAL_ON_POOL = True  # out = num*rden on gpsimd (ACT evacuates num first)

_STATE = {}


def _patch_act_tables():
    """Force Exp and Ln to resolve to the one ACT table set containing both,
    so the kernel pays a single ACT_TABLE_LOAD instead of thrashing."""
    import concourse.bacc as bacc_mod
    import concourse.mybir as mybir
    from concourse.hw_specs import get_activation_tables as orig

    AF = mybir.ActivationFunctionType

    def patched(arch):
        out = {}
        for name, funcs in orig(arch).items():
            f = set(funcs)
            if name != "natural_log_exp_and_others":
                f.discard(AF.Exp)
                f.discard(AF.Ln)
            out[name] = f
        return out

    bacc_mod.get_activation_tables = patched


def _build_nc():
    import concourse.bass as bass
    import concourse.tile as tile
    from concourse import bacc, mybir

    _patch_act_tables()
    _STATE.pop("pe_wkey", None)

    bf16 = mybir.dt.bfloat16
    i16 = mybir.dt.int16
    f32 = mybir.dt.float32
    AF = mybir.ActivationFunctionType
    ALU = mybir.AluOpType

    nc = bacc.Bacc("TRN2", target_bir_lowering=False, debug=False, num_devices=8)

    x_d = nc.declare_dram_parameter("x", [C, W], bf16, isOutput=False)
    w_d = {
        t: nc.declare_dram_parameter(f"wt{t}", [C, C], bf16, isOutput=False)
        for t in "qkv"
    }
    id_d = nc.declare_dram_parameter("ident", [128, 128], bf16, isOutput=False)
    out_d = nc.declare_dram_parameter("out", [C, W], bf16, isOutput=True)

    n_gg = W // GG  # gemm groups per co block (4)
    n_sc = W // SC  # sum chunks per co block (4)

    with tile.TileContext(nc) as tc:
        from contextlib import ExitStack

        with ExitStack() as ctx:
            persist = ctx.enter_context(tc.tile_pool(name="persist", bufs=1))
            lpool = ctx.enter_context(tc.tile_pool(name="lpool", bufs=2))
            rpool = ctx.enter_context(tc.tile_pool(name="rpool", bufs=3))
            npool = ctx.enter_context(tc.tile_pool(name="npool", bufs=3))
            opool = ctx.enter_context(tc.tile_pool(name="opool", bufs=3))

            # ---- persistent SBUF tensors ----
            xb = persist.tile([128, 2, W], bf16, tag="xb")  # x, ci-major blocks
            wsb = {
                t: persist.tile([128, 2, C], bf16, name=f"wsb_{t}", tag=f"wsb_{t}")
                for t in "qkv"
            }  # w.T
            idt = persist.tile([128, 128], bf16, tag="idt")
            qsb = persist.tile([128, 2, W], bf16, tag="qsb")
            ksb = persist.tile([128, 2, WP], bf16, tag="ksb")
            vsb = persist.tile([128, 2, WP], bf16, tag="vsb")
            # score/e/ev planes, both co blocks (ev done in place over e)
            st = persist.tile([128, 2, K7, W], bf16, tag="st")

            # ---- loads ----
            for cb in range(2):
                nc.sync.dma_start(
                    out=wsb["q"][:, cb, :], in_=w_d["q"][cb * 128 : (cb + 1) * 128, :]
                )
            nc.sync.dma_start(out=idt[:, :], in_=id_d[:, :])
            for cb in range(2):
                nc.sync.dma_start(
                    out=xb[:, cb, :], in_=x_d[cb * 128 : (cb + 1) * 128, :]
                )
            for t in "kv":
                for cb in range(2):
                    nc.sync.dma_start(
                        out=wsb[t][:, cb, :], in_=w_d[t][cb * 128 : (cb + 1) * 128, :]
                    )

            # zero the pad columns of k and v
            for buf in (ksb, vsb):
                for cb in range(2):
                    nc.vector.memset(buf[:, cb, 0:PAD], 0.0)
                    nc.vector.memset(buf[:, cb, W + PAD : WP], 0.0)

            def mm(out, lhsT, rhs, start, stop, wkey, epoch=False, **kw):
                """plain matmul; redundant LDWEIGHTS are removed post-schedule
                by _dedupe_ldweights (wkey/epoch kept for readability)."""
                return nc.tensor.matmul(out, lhsT, rhs, start=start, stop=stop, **kw)

            def warmup(gpsum):
                """PE warmup burst: dummy matmuls on the wq tiles so the pstate
                ramp + HAM clock-gate release before the real GEMM stream."""
                wps = gpsum.tile([128, GG], f32, name="wps", tag="gps")
                for i in range(16):
                    mm(
                        wps[:, 0:256],
                        wsb["q"][:, 0, 0:128],
                        wsb["q"][:, i % 2, :],
                        start=True,
                        stop=True,
                        wkey="warm",
                        skip_group_check=True,
                    )

            def gemm_group(co, g, t, gpsum):
                """GEMM of tensor t for output cols [g*GG, (g+1)*GG) of
                co-block, one PSUM tile + one ACT evacuation. ci-outer so the
                two 512-halves of a group share one weight load."""
                co_sl = slice(co * 128, (co + 1) * 128)
                ps = gpsum.tile([128, GG], f32, name="ps", tag="gps")
                for ci in range(2):
                    for i in range(GG // 512):
                        w0 = g * GG + i * 512
                        mm(
                            ps[:, i * 512 : (i + 1) * 512],
                            wsb[t][:, ci, co_sl],
                            xb[:, ci, w0 : w0 + 512],
                            start=(ci == 0),
                            stop=(ci == 1),
                            wkey=("w", t, co, ci),
                            epoch=(ci == 0 and i == 0),
                        )
                if t == "q":
                    dst = qsb[:, co, g * GG : (g + 1) * GG]
                else:
                    buf = ksb if t == "k" else vsb
                    dst = buf[:, co, PAD + g * GG : PAD + (g + 1) * GG]
                nc.scalar.copy(out=dst, in_=ps[:, :])

            def q_bc(co, j0, nj):
                qsl = qsb[:, co, :]
                return bass.AP(
                    tensor=qsl.tensor,
                    offset=qsl.offset,
                    ap=[qsl.ap[0], [0, nj], [1, W]],
                )

            def k_win(co, j0, nj):
                ksl = ksb[:, co, :]
                return bass.AP(
                    tensor=ksl.tensor,
                    offset=ksl.offset + j0,
                    ap=[ksl.ap[0], [1, nj], [1, W]],
                )

            def v_win(co, j0, nj):
                vsl = vsb[:, co, :]
                return bass.AP(
                    tensor=vsl.tensor,
                    offset=vsl.offset + j0,
                    ap=[vsl.ap[0], [1, nj], [1, W]],
                )

            def scores_exp(co):
                """s_j = q*k_j then e = exp(s) in place, split in j-halves so
                exp overlaps the second score mult."""
                for j0, nj in ((0, 4), (4, 3)):
                    dst = st[:, co, j0 : j0 + nj, :]
                    nc.vector.tensor_tensor(
                        dst, q_bc(co, j0, nj), k_win(co, j0, nj), ALU.mult
                    )
                    a_lo = max(j0, 0)
                    a_hi = min(j0 + nj, N_ACT_PLANES)
                    if a_hi > a_lo:  # exact ACT planes
                        sl = st[:, co, a_lo:a_hi, :]
                        nc.scalar.activation(sl, sl, AF.Exp)
                    s_lo = max(j0, N_ACT_PLANES)
                    s_hi = j0 + nj
                    if s_hi > s_lo:  # Schraudolph planes on DVE (4x)
                        sl = st[:, co, s_lo:s_hi, :]
                        nc.vector.tensor_scalar(
                            sl.bitcast(i16),
                            sl,
                            SCHRAUD_C0,
                            SCHRAUD_C1,
                            ALU.mult,
                            ALU.add,
                        )

            def ev_mult(co, wh):
                """ev_j = e_j * v_j in place for w-half wh (PE den sums must
                already have consumed the e values for this co block). W-split
                so the num sums of the first half start before the second."""
                w0 = wh * (W // 2)
                sl = st[:, co, :, w0 : w0 + W // 2]
                vsl = vsb[:, co, :]
                vw = bass.AP(
                    tensor=vsl.tensor,
                    offset=vsl.offset + w0,
                    ap=[vsl.ap[0], [1, K7], [1, W // 2]],
                )
                nc.vector.tensor_tensor(sl, sl, vw, ALU.mult)

            def pe_sum(co, m, pool, tag):
                """7-plane sum over j for w-cols [m*SC, (m+1)*SC) via identity
                matmuls accumulating in PSUM. Returns the PSUM tile."""
                ps = pool.tile([128, SC], f32, name=tag, tag=tag)
                for h in range(SC // 512):
                    w0 = m * SC + h * 512
                    for j in range(K7):
                        mm(
                            ps[:, h * 512 : (h + 1) * 512],
                            idt[:, :],
                            st[:, co, j, w0 : w0 + 512],
                            start=(j == 0),
                            stop=(j == K7 - 1),
                            wkey="ident",
                            epoch=(j == 0),
                        )
                return ps

            def rden_of(denp):
                """rden = exp(-ln(den)) on ACT; ln kept fp32 to avoid bf16
                ulp noise on large |ln den|."""
                t = lpool.tile([128, SC], f32, name="lnt", tag="lnt")
                r = rpool.tile([128, SC], bf16, name="rd", tag="rd")
                nc.scalar.activation(t[:, :], denp[:, :], AF.Ln)
                nc.scalar.activation(r[:, :], t[:, :], AF.Exp, scale=-1.0)
                return r

            def final_out(co, m, nump, rd):
                """out = num * rden -> bf16, DMA to HBM. Either DVE reads num
                straight from PSUM (1x), or ACT evacuates num to SBUF and the
                idle gpsimd does the multiply."""
                co_sl = slice(co * 128, (co + 1) * 128)
                w0 = m * SC
                oc = opool.tile([128, SC], bf16, name="oc", tag="oc")
                if FINAL_ON_POOL:
                    ns = npool.tile([128, SC], bf16, name="ns", tag="ns")
                    nc.scalar.copy(out=ns[:, :], in_=nump[:, :])
                    nc.gpsimd.tensor_tensor(oc[:, :], ns[:, :], rd[:, :], ALU.mult)
                else:
                    nc.vector.tensor_tensor(oc[:, :], nump[:, :], rd[:, :], ALU.mult)
                nc.sync.dma_start(out=out_d[co_sl, w0 : w0 + SC], in_=oc[:, :])

            # ---- emission ----
            # ACT queue order is emission order, so interleave per co-block:
            # q/k GEMM evacs, then this block's exps, then the v evacs — the
            # exp for co-block 0 must not sit behind co-block 1's evacuations.
            with tc.tile_pool(name="gpsum", bufs=3, space="PSUM") as gpsum:
                warmup(gpsum)
                for co in range(2):
                    for t in "qkv":
                        for g in range(n_gg):
                            gemm_group(co, g, t, gpsum)
                    scores_exp(co)
            with (
                tc.tile_pool(name="dpsum", bufs=2, space="PSUM") as dpsum,
                tc.tile_pool(name="npsum", bufs=2, space="PSUM") as npsum,
            ):
                dens = {}
                for co in range(2):
                    for m in range(n_sc):
                        dens[(co, m)] = pe_sum(co, m, dpsum, "den")
                for co in range(2):
                    for wh in range(2):
                        ev_mult(co, wh)
                for co in range(2):
                    for m in range(n_sc):
                        rd = rden_of(dens[(co, m)])  # ACT; frees den tile
                        nump = pe_sum(co, m, npsum, "num")
                        final_out(co, m, nump, rd)

    if LDW_SKIP:
        _dedupe_ldweights(nc, mybir)
    nc.finalize()
    return nc


def _dedupe_ldweights(nc, mybir):
    """Remove redundant InstLdweights: a reload of the exact weights already
    resident in the PE array. The tile scheduler splits every matmul into
    LDWEIGHTS + MATMUL; back-to-back same-weight matmuls then pay a ~107ns
    reload plus a lost drain/fill overlap (~166ns) each. Only drops loads
    that carry no semaphore waits/updates, so sync is untouched; any other
    PE instruction type resets the tracked signature."""

    def wsig(ldw):
        return (
            str(ldw.ins[0]),
            str(ldw.is_transpose),
            str(ldw.perf_mode),
            str(ldw.tile_position),
        )

    removed = 0
    for f in nc.m.functions:
        for b in f.blocks:
            keep = []
            last = None
            for i in b.instructions:
                tn = type(i).__name__
                if getattr(i, "engine", None) == mybir.EngineType.PE:
                    if tn == "InstLdweights":
                        si = i.sync_info
                        clean = si is None or (
                            len(si.on_wait) == 0 and len(si.on_update) == 0
                        )
                        if clean and last == wsig(i):
                            removed += 1
                            continue
                        last = wsig(i)
                    elif tn in ("InstMatmult", "InstEventSemaphore"):
                        pass
                    else:
                        last = None
                keep.append(i)
            b.instructions[:] = keep
    return removed


def _get_nc():
    if "nc" not in _STATE:
        _STATE["nc"] = _build_nc()
    return _STATE["nc"]


def _make_in_maps(x, wq, wk, wv):
    import ml_dtypes

    bf = ml_dtypes.bfloat16

    x = np.asarray(x, dtype=np.float32)
    wqT = np.ascontiguousarray(np.asarray(wq, dtype=np.float32).T).astype(bf)
    wkT = np.ascontiguousarray(np.asarray(wk, dtype=np.float32).T).astype(bf)
    wvT = np.ascontiguousarray(np.asarray(wv, dtype=np.float32).T).astype(bf)
    xb = x.astype(bf)
    ident = np.eye(128, dtype=np.float32).astype(bf)

    return [
        {
            "x": np.ascontiguousarray(xb[b]),
            "wtq": wqT,
            "wtk": wkT,
            "wtv": wvT,
            "ident": ident,
        }
        for b in range(B)
    ]


def kernel(x, wq, wk, wv):
    nc = _get_nc()
    in_maps = _make_in_maps(x, wq, wk, wv)

    from concourse.bass_utils import run_bass_kernel_spmd

    res = run_bass_kernel_spmd(nc, in_maps, core_ids=list(range(B)))
    outs = [np.asarray(res.results[i]["out"], dtype=np.float32) for i in range(B)]
    return np.stack(outs)


# revision 23
# speedup vs baseline: 1.2698x; 1.0068x over previous
"""AttentionConv kernel for Trainium2 (8 NeuronCores, SPMD data-parallel over batch).

Problem: per-channel windowed softmax attention.
  q = wq @ x; k = wk @ pad(x, 3); v = wv @ pad(x, 3)       (1x1 convs = GEMMs)
  s_j[c,w] = q[c,w] * k[c,w+j],  j = 0..6
  out[c,w] = sum_j softmax_j(s)[c,w,j] * v[c,w+j]

Sharding: batch B=8 -> one batch element per core; weights replicated.

v2 engine mapping (vs v1 which ran everything elementwise on DVE/ACT):
  TensorE: q/k/v GEMMs (bf16) AND the two 7-plane window reductions
           (den = sum_j e_j, num = sum_j e_j*v_j) as identity-weight
           matmuls accumulating in PSUM fp32. Keeps PE busy -> HAM stays
           at 8/8 (2.4 GHz) instead of the 4/8 throttle v1 suffered.
  VectorE: score mults and e*v mults (bf16 2x, full-width 4096 rows to
           amortize the ~235-cycle per-row bubble), exp via Schraudolph
           bit-trick tensor_scalar -> int16 (4x mode), final
           out = num(PSUM) * rden.
  ScalarE: PSUM->SBUF GEMM evacuation casts, optional exact exp planes,
           rden = exp(-ln(den)) pinned to the one ACT table set with both.
  Host upcasts the bf16 output to fp32.
"""

import sys

sys.path.insert(0, "/opt/trn_rl_repo")

import numpy as np

B, C, W = 8, 256, 4096
K7, PAD = 7, 3
WP = W + 2 * PAD
GG = 1024  # gemm / psum evac group (2 PSUM banks)
SC = 1024  # sum-chunk width for den/num PSUM accumulators

# --- tuning knobs -----------------------------------------------------------
N_ACT_PLANES = 0  # j-planes [0, n) get exact ACT exp; rest Schraudolph on DVE
SCHRAUD_C0 = 184.6650390625  # 2^7 / ln 2
SCHRAUD_C1 = 16250.0  # 127 * 128 - sigma
LDW_SKIP = True  # non-self-loading matmuls when PE weights are unchanged
FINAL_ON_POOL = False  # out = num*rden on gpsimd (ACT evacuates num first)

_STATE = {}


def _patch_act_tables():
    """Force Exp and Ln to resolve to the one ACT table set containing both,
    so the kernel pays a single ACT_TABLE_LOAD instead of thrashing."""
    import concourse.bacc as bacc_mod
    import concourse.mybir as mybir
    from concourse.hw_specs import get_activation_tables as orig

    AF = mybir.ActivationFunctionType

    def patched(arch):
        out = {}
        for name, funcs in orig(arch).items():
            f = set(funcs)
            if name != "natural_log_exp_and_others":
                f.discard(AF.Exp)
                f.discard(AF.Ln)
            out[name] = f
        return out

    bacc_mod.get_activation_tables = patched


def _build_nc():
    import concourse.bass as bass
    import concourse.tile as tile
    from concourse import bacc, mybir

    _patch_act_tables()
    _STATE.pop("pe_wkey", None)

    bf16 = mybir.dt.bfloat16
    i16 = mybir.dt.int16
    f32 = mybir.dt.float32
    AF = mybir.ActivationFunctionType
    ALU = mybir.AluOpType

    nc = bacc.Bacc("TRN2", target_bir_lowering=False, debug=False, num_devices=8)

    x_d = nc.declare_dram_parameter("x", [C, W], bf16, isOutput=False)
    w_d = {
        t: nc.declare_dram_parameter(f"wt{t}", [C, C], bf16, isOutput=False)
        for t in "qkv"
    }
    id_d = nc.declare_dram_parameter("ident", [128, 128], bf16, isOutput=False)
    out_d = nc.declare_dram_parameter("out", [C, W], bf16, isOutput=True)

    n_gg = W // GG  # gemm groups per co block (4)
    n_sc = W // SC  # sum chunks per co block (4)

    with tile.TileContext(nc) as tc:
        from contextlib import ExitStack

        with ExitStack() as ctx:
            persist = ctx.enter_context(tc.tile_pool(name="persist", bufs=1))
            lpool = ctx.enter_context(tc.tile_pool(name="lpool", bufs=2))
            rpool = ctx.enter_context(tc.tile_pool(name="rpool", bufs=3))
            npool = ctx.enter_context(tc.tile_pool(name="npool", bufs=3))
            opool = ctx.enter_context(tc.tile_pool(name="opool", bufs=3))

            # ---- persistent SBUF tensors ----
            xb = persist.tile([128, 2, W], bf16, tag="xb")  # x, ci-major blocks
            wsb = {
                t: persist.tile([128, 2, C], bf16, name=f"wsb_{t}", tag=f"wsb_{t}")
                for t in "qkv"
            }  # w.T
            idt = persist.tile([128, 128], bf16, tag="idt")
            qsb = persist.tile([128, 2, W], bf16, tag="qsb")
            ksb = persist.tile([128, 2, WP], bf16, tag="ksb")
            vsb = persist.tile([128, 2, WP], bf16, tag="vsb")
            # score/e/ev planes, both co blocks (ev done in place over e)
            st = persist.tile([128, 2, K7, W], bf16, tag="st")

            # ---- loads ----
            for cb in range(2):
                nc.sync.dma_start(
                    out=wsb["q"][:, cb, :], in_=w_d["q"][cb * 128 : (cb + 1) * 128, :]
                )
            nc.sync.dma_start(out=idt[:, :], in_=id_d[:, :])
            for cb in range(2):
                nc.sync.dma_start(
                    out=xb[:, cb, :], in_=x_d[cb * 128 : (cb + 1) * 128, :]
                )
            for t in "kv":
                for cb in range(2):
                    nc.sync.dma_start(
                        out=wsb[t][:, cb, :], in_=w_d[t][cb * 128 : (cb + 1) * 128, :]
                    )

            # zero the pad columns of k and v
            for buf in (ksb, vsb):
                for cb in range(2):
                    nc.vector.memset(buf[:, cb, 0:PAD], 0.0)
                    nc.vector.memset(buf[:, cb, W + PAD : WP], 0.0)

            def mm(out, lhsT, rhs, start, stop, wkey, epoch=False, **kw):
                """plain matmul; redundant LDWEIGHTS are removed post-schedule
                by _dedupe_ldweights (wkey/epoch kept for readability)."""
                return nc.tensor.matmul(out, lhsT, rhs, start=start, stop=stop, **kw)

            def warmup(gpsum):
                """PE warmup burst: dummy matmuls on the wq tiles so the pstate
                ramp + HAM clock-gate release before the real GEMM stream."""
                wps = gpsum.tile([128, GG], f32, name="wps", tag="gps")
                for i in range(16):
                    mm(
                        wps[:, 0:256],
                        wsb["q"][:, 0, 0:128],
                        wsb["q"][:, i % 2, :],
                        start=True,
                        stop=True,
                        wkey="warm",
                        skip_group_check=True,
                    )

            def gemm_group(co, g, t, gpsum):
                """GEMM of tensor t for output cols [g*GG, (g+1)*GG) of
                co-block, one PSUM tile + one ACT evacuation. ci-outer so the
                two 512-halves of a group share one weight load."""
                co_sl = slice(co * 128, (co + 1) * 128)
                ps = gpsum.tile([128, GG], f32, name="ps", tag="gps")
                for ci in range(2):
                    for i in range(GG // 512):
                        w0 = g * GG + i * 512
                        mm(
                            ps[:, i * 512 : (i + 1) * 512],
                            wsb[t][:, ci, co_sl],
                            xb[:, ci, w0 : w0 + 512],
                            start=(ci == 0),
                            stop=(ci == 1),
                            wkey=("w", t, co, ci),
                            epoch=(ci == 0 and i == 0),
                        )
                if t == "q":
                    dst = qsb[:, co, g * GG : (g + 1) * GG]
                else:
                    buf = ksb if t == "k" else vsb
                    dst = buf[:, co, PAD + g * GG : PAD + (g + 1) * GG]
                nc.scalar.copy(out=dst, in_=ps[:, :])

            def q_bc(co, j0, nj):
                qsl = qsb[:, co, :]
                return bass.AP(
                    tensor=qsl.tensor,
                    offset=qsl.offset,
                    ap=[qsl.ap[0], [0, nj], [1, W]],
                )

            def k_win(co, j0, nj):
                ksl = ksb[:, co, :]
                return bass.AP(
                    tensor=ksl.tensor,
                    offset=ksl.offset + j0,
                    ap=[ksl.ap[0], [1, nj], [1, W]],
                )

            def v_win(co, j0, nj):
                vsl = vsb[:, co, :]
                return bass.AP(
                    tensor=vsl.tensor,
                    offset=vsl.offset + j0,
                    ap=[vsl.ap[0], [1, nj], [1, W]],
                )

            def scores_exp(co):
                """s_j = q*k_j then e = exp(s) in place, split in j-halves so
                exp overlaps the second score mult."""
                for j0, nj in ((0, 4), (4, 3)):
                    dst = st[:, co, j0 : j0 + nj, :]
                    nc.vector.tensor_tensor(
                        dst, q_bc(co, j0, nj), k_win(co, j0, nj), ALU.mult
                    )
                    a_lo = max(j0, 0)
                    a_hi = min(j0 + nj, N_ACT_PLANES)
                    if a_hi > a_lo:  # exact ACT planes
                        sl = st[:, co, a_lo:a_hi, :]
                        nc.scalar.activation(sl, sl, AF.Exp)
                    s_lo = max(j0, N_ACT_PLANES)
                    s_hi = j0 + nj
                    if s_hi > s_lo:  # Schraudolph planes on DVE (4x)
                        sl = st[:, co, s_lo:s_hi, :]
                        nc.vector.tensor_scalar(
                            sl.bitcast(i16),
                            sl,
                            SCHRAUD_C0,
                            SCHRAUD_C1,
                            ALU.mult,
                            ALU.add,
                        )

            def ev_mult(co, wh):
                """ev_j = e_j * v_j in place for w-half wh (PE den sums must
                already have consumed the e values for this co block). W-split
                so the num sums of the first half start before the second."""
                w0 = wh * (W // 2)
                sl = st[:, co, :, w0 : w0 + W // 2]
                vsl = vsb[:, co, :]
                vw = bass.AP(
                    tensor=vsl.tensor,
                    offset=vsl.offset + w0,
                    ap=[vsl.ap[0], [1, K7], [1, W // 2]],
                )
                nc.vector.tensor_tensor(sl, sl, vw, ALU.mult)

            def pe_sum(co, m, pool, tag):
                """7-plane sum over j for w-cols [m*SC, (m+1)*SC) via identity
                matmuls accumulating in PSUM. Returns the PSUM tile."""
                ps = pool.tile([128, SC], f32, name=tag, tag=tag)
                for h in range(SC // 512):
                    w0 = m * SC + h * 512
                    for j in range(K7):
                        mm(
                            ps[:, h * 512 : (h + 1) * 512],
                            idt[:, :],
                            st[:, co, j, w0 : w0 + 512],
                            start=(j == 0),
                            stop=(j == K7 - 1),
                            wkey="ident",
                            epoch=(j == 0),
                        )
                return ps

            def rden_of(denp):
                """rden = exp(-ln(den)) on ACT; ln kept fp32 to avoid bf16
                ulp noise on large |ln den|."""
                t = lpool.tile([128, SC], f32, name="lnt", tag="lnt")
                r = rpool.tile([128, SC], bf16, name="rd", tag="rd")
                nc.scalar.activation(t[:, :], denp[:, :], AF.Ln)
                nc.scalar.activation(r[:, :], t[:, :], AF.Exp, scale=-1.0)
                return r

            def final_out(co, m, nump, rd):
                """out = num * rden -> bf16, DMA to HBM. Either DVE reads num
                straight from PSUM (1x), or ACT evacuates num to SBUF and the
                idle gpsimd does the multiply."""
                co_sl = slice(co * 128, (co + 1) * 128)
                w0 = m * SC
                oc = opool.tile([128, SC], bf16, name="oc", tag="oc")
                if FINAL_ON_POOL:
                    ns = npool.tile([128, SC], bf16, name="ns", tag="ns")
                    nc.scalar.copy(out=ns[:, :], in_=nump[:, :])
                    nc.gpsimd.tensor_tensor(oc[:, :], ns[:, :], rd[:, :], ALU.mult)
                else:
                    nc.vector.tensor_tensor(oc[:, :], nump[:, :], rd[:, :], ALU.mult)
                nc.sync.dma_start(out=out_d[co_sl, w0 : w0 + SC], in_=oc[:, :])

            # ---- emission ----
            # ACT queue order is emission order, so interleave per co-block:
            # q/k GEMM evacs, then this block's exps, then the v evacs — the
            # exp for co-block 0 must not sit behind co-block 1's evacuations.
            with tc.tile_pool(name="gpsum", bufs=3, space="PSUM") as gpsum:
                warmup(gpsum)
                for co in range(2):
                    for t in "qkv":
                        for g in range(n_gg):
                            gemm_group(co, g, t, gpsum)
                    scores_exp(co)
            with (
                tc.tile_pool(name="dpsum", bufs=2, space="PSUM") as dpsum,
                tc.tile_pool(name="npsum", bufs=2, space="PSUM") as npsum,
            ):
                dens = {}
                for co in range(2):
                    for m in range(n_sc):
                        dens[(co, m)] = pe_sum(co, m, dpsum, "den")

                def tail(co, m):
                    rd = rden_of(dens[(co, m)])  # ACT; frees den tile
                    nump = pe_sum(co, m, npsum, "num")
                    final_out(co, m, nump, rd)

                # interleave co0's normalize/final work between co1's ev
                # halves so the DVE queue never parks finals behind all evs
                ev_mult(0, 0)
                ev_mult(0, 1)
                ev_mult(1, 0)
                tail(0, 0)
                tail(0, 1)
                ev_mult(1, 1)
                tail(0, 2)
                tail(0, 3)
                for m in range(n_sc):
                    tail(1, m)

    if LDW_SKIP:
        _dedupe_ldweights(nc, mybir)
    nc.finalize()
    return nc


def _dedupe_ldweights(nc, mybir):
    """Remove redundant InstLdweights: a reload of the exact weights already
    resident in the PE array. The tile scheduler splits every matmul into
    LDWEIGHTS + MATMUL; back-to-back same-weight matmuls then pay a ~107ns
    reload plus a lost drain/fill overlap (~166ns) each. Only drops loads
    that carry no semaphore waits/updates, so sync is untouched; any other
    PE instruction type resets the tracked signature."""

    def wsig(ldw):
        return (
            str(ldw.ins[0]),
            str(ldw.is_transpose),
            str(ldw.perf_mode),
            str(ldw.tile_position),
        )

    removed = 0
    for f in nc.m.functions:
        for b in f.blocks:
            keep = []
            last = None
            for i in b.instructions:
                tn = type(i).__name__
                if getattr(i, "engine", None) == mybir.EngineType.PE:
                    if tn == "InstLdweights":
                        si = i.sync_info
                        clean = si is None or (
                            len(si.on_wait) == 0 and len(si.on_update) == 0
                        )
                        if clean and last == wsig(i):
                            removed += 1
                            continue
                        last = wsig(i)
                    elif tn in ("InstMatmult", "InstEventSemaphore"):
                        pass
                    else:
                        last = None
                keep.append(i)
            b.instructions[:] = keep
    return removed


def _get_nc():
    if "nc" not in _STATE:
        _STATE["nc"] = _build_nc()
    return _STATE["nc"]


def _make_in_maps(x, wq, wk, wv):
    import ml_dtypes

    bf = ml_dtypes.bfloat16

    x = np.asarray(x, dtype=np.float32)
    wqT = np.ascontiguousarray(np.asarray(wq, dtype=np.float32).T).astype(bf)
    wkT = np.ascontiguousarray(np.asarray(wk, dtype=np.float32).T).astype(bf)
    wvT = np.ascontiguousarray(np.asarray(wv, dtype=np.float32).T).astype(bf)
    xb = x.astype(bf)
    ident = np.eye(128, dtype=np.float32).astype(bf)

    return [
        {
            "x": np.ascontiguousarray(xb[b]),
            "wtq": wqT,
            "wtk": wkT,
            "wtv": wvT,
            "ident": ident,
        }
        for b in range(B)
    ]


def kernel(x, wq, wk, wv):
    nc = _get_nc()
    in_maps = _make_in_maps(x, wq, wk, wv)

    from concourse.bass_utils import run_bass_kernel_spmd

    res = run_bass_kernel_spmd(nc, in_maps, core_ids=list(range(B)))
    outs = [np.asarray(res.results[i]["out"], dtype=np.float32) for i in range(B)]
    return np.stack(outs)
